# revision 9
# baseline (speedup 1.0000x reference)
"""Trainium2 Bass kernel for LoRA-augmented causal attention.

Reference computation (per nn_Attention_31688268710508):
  x:(B,S,D) -> q/k/v = x@W* + broadcast LoRA + shared head-offset LoRA,
  RoPE(q,k), causal softmax attention per (b,head), out-proj with wo.

Strategy (8 NeuronCores, tensor-parallel over heads):
  * All rank-8 LoRA terms are rank-8 matrices in the weight space, so they are
    folded into effective projection weights on the host (x @ (A@B) == (x@A)@B
    up to fp rounding).  The softmax 1/sqrt(HD) scale is folded into Wq.
  * RoPE pairs (2i,2i+1) are moved to (i, i+64) by permuting Wq/Wk columns
    (scores are invariant to a shared permutation of q/k head dims), making
    RoPE a half-partition-block rotation in the on-chip [hd, s] layout.
  * Each core projects QT/KT/V for its 2 heads (W stationary, x^T streamed),
    runs causal attention with S^T = K^T.T @ Q^T blocks, exp -> fp16 P^T,
    and A@V via P^T-stationary matmuls against V augmented with a ones
    column (giving the softmax denominator for free).
  * Attention outputs are transposed on the PE and exchanged with an
    AllToAll so each core owns 512 sequence rows with all 2048 head dims,
    then multiplies with the full wo to produce its output row shard.
"""

import math
import os
import sys

import numpy as np

for _p in ("/opt/trn_rl_repo", "/root/.axon_site/_ro/trn_rl_repo"):
    if os.path.isdir(_p) and _p not in sys.path:
        sys.path.insert(0, _p)

import concourse.bass as bass  # noqa: E402
from concourse import bacc  # noqa: E402
import concourse.mybir as mybir  # noqa: E402
import concourse.tile as tile  # noqa: E402
from concourse.masks import make_identity  # noqa: E402

F32 = mybir.dt.float32
F32R = mybir.dt.float32r
F16 = mybir.dt.float16

NEG = -1.0e30


class Cfg:
    def __init__(self, B=2, S=2048, D=2048, HEADS=16, NCORES=8):
        self.B, self.S, self.D, self.NCORES = B, S, D, NCORES
        self.HD = 128
        self.HPC = HEADS // NCORES          # heads per core
        self.BS = B * S
        self.RPC = self.BS // NCORES        # output rows per core
        self.ST = 512                       # free-dim tile (q tile, s tile)
        self.SG = min(1024, self.BS)        # x^T slab width (s cols per slab)
        self.NDB = D // 128                 # d blocks (contraction)
        self.NSG = self.BS // self.SG
        self.NST_G = self.SG // self.ST     # s tiles per slab
        self.NQT = S // self.ST             # q tiles per (b, h)
        self.NKB = S // 128                 # k blocks per (b, h)
        self.NHB = (HEADS * self.HD) // 128  # head-dim blocks, total (out-proj K)
        assert self.HD == 128 and D % 512 == 0 and S % self.ST == 0
        assert self.RPC % 128 == 0 and self.SG % self.ST == 0


def build_bass(cfg: Cfg) -> bass.Bass:
    B, S, D = cfg.B, cfg.S, cfg.D
    HPC, HD, ST, SG = cfg.HPC, cfg.HD, cfg.ST, cfg.SG
    NC_, RPC = cfg.NCORES, cfg.RPC

    nc = bacc.Bacc("TRN2", target_bir_lowering=False, debug=False,
                   num_devices=NC_)

    xT = nc.declare_dram_parameter("xT", [D, cfg.BS], F16, isOutput=False)
    wq = nc.declare_dram_parameter("wq", [D, HPC * HD], F16, isOutput=False)
    wk = nc.declare_dram_parameter("wk", [D, HPC * HD], F16, isOutput=False)
    wv = nc.declare_dram_parameter("wv", [D, HPC * HD], F16, isOutput=False)
    wo = nc.declare_dram_parameter("wo", [NC_ * HPC * HD, D], F16,
                                   isOutput=False)
    cosT = nc.declare_dram_parameter("cosT", [128, S], F32, isOutput=False)
    sinT = nc.declare_dram_parameter("sinT", [128, S], F32, isOutput=False)
    band = nc.declare_dram_parameter("band", [128, 2 * ST - 128], F32,
                                     isOutput=False)
    out_rows = nc.declare_dram_parameter("out_rows", [RPC, D], F32,
                                         isOutput=True)

    a2a_in = nc.dram_tensor("a2a_in", [NC_, HPC * HD, RPC], F16)
    a2a_out = nc.dram_tensor("a2a_out", [NC_, HPC * HD, RPC], F16)

    with tile.TileContext(nc) as tc:
        with tc.tile_pool(name="const", bufs=1) as constp:
            ident = constp.tile([128, 128], F16)
            make_identity(nc, ident)
            band_sb = constp.tile([128, 2 * ST - 128], F32)
            nc.sync.dma_start(out=band_sb, in_=band[:, :])

            with (
                tc.tile_pool(name="qt", bufs=1) as qtp,
                tc.tile_pool(name="kt", bufs=1) as ktp,
                tc.tile_pool(name="vhat", bufs=1) as vhp,
            ):
                QT = qtp.tile([128, HPC, cfg.BS], F16)
                KT = ktp.tile([128, HPC, cfg.BS], F16)
                Vhat = vhp.tile([128, HPC, B, S // 128, 130], F16)
                for h in range(HPC):
                    for b in range(B):
                        nc.vector.memset(Vhat[:, h, b, :, 128:130], 1.0)

                _projection_phase(nc, tc, cfg, xT, wq, wk, wv, cosT, sinT,
                                  QT, KT, Vhat, ident)
                _attention_phase(nc, tc, cfg, QT, KT, Vhat, band_sb, ident,
                                 a2a_in)

            nc.gpsimd.collective_compute(
                "AllToAll",
                mybir.AluOpType.bypass,
                replica_groups=[list(range(NC_))],
                ins=[a2a_in[:, :, :]],
                outs=[a2a_out[:, :, :]],
            )

            _outproj_phase(nc, tc, cfg, a2a_out, wo, out_rows)

    nc.finalize()
    return nc


def _projection_phase(nc, tc, cfg, xT, wq, wk, wv, cosT, sinT, QT, KT, Vhat,
                      ident):
    """QT/KT (RoPE'd, [hd, s] layout) and Vhat ([s, hd]+ones, fp16)."""
    B, S = cfg.B, cfg.S
    HPC, ST, SG = cfg.HPC, cfg.ST, cfg.SG
    NDB, NSG, NST_G = cfg.NDB, cfg.NSG, cfg.NST_G
    w_drams = [wq, wk, wv]

    with (
        tc.tile_pool(name="xslab", bufs=NDB + 3) as xp,
        tc.tile_pool(name="wbuf", bufs=4) as wp,
        tc.tile_pool(name="tables", bufs=1) as tbp,
        tc.tile_pool(name="ropet", bufs=4) as rp,
        tc.tile_pool(name="vstage", bufs=4) as stp,
        tc.tile_pool(name="pacc", bufs=6, space="PSUM") as pap,
        tc.tile_pool(name="ptrans", bufs=2, space="PSUM") as ptp,
    ):
        cos_sb = tbp.tile([128, S], F32)
        nc.sync.dma_start(out=cos_sb, in_=cosT[:, :])
        sin_sb = tbp.tile([128, S], F32)
        nc.sync.dma_start(out=sin_sb, in_=sinT[:, :])

        for g in range(NSG):
            xs = []
            for db in range(NDB):
                t = xp.tile([128, SG], F16, tag="xs")
                nc.sync.dma_start(
                    out=t, in_=xT[db * 128:(db + 1) * 128,
                                  g * SG:(g + 1) * SG])
                xs.append(t)

            for proj in range(3):
                psums = [[pap.tile([128, ST], F32, tag="pacc",
                                   name=f"pacc_{h}_{st}")
                          for st in range(NST_G)] for h in range(HPC)]
                for db in range(NDB):
                    w_t = wp.tile([128, HPC * 128], F16, tag="w")
                    nc.sync.dma_start(
                        out=w_t, in_=w_drams[proj][db * 128:(db + 1) * 128, :])
                    for h in range(HPC):
                        for st in range(NST_G):
                            nc.tensor.matmul(
                                psums[h][st],
                                lhsT=w_t[:, h * 128:(h + 1) * 128],
                                rhs=xs[db][:, st * ST:(st + 1) * ST],
                                start=(db == 0),
                                stop=(db == NDB - 1),
                            )
                for h in range(HPC):
                    for st in range(NST_G):
                        gcol = g * SG + st * ST     # column in [0, B*S)
                        scol = gcol % S             # column in rope tables
                        acc = psums[h][st]
                        if proj < 2:
                            # RoPE: rows 0:64 = "real", 64:128 = "imag".
                            dst = (QT if proj == 0 else KT)[:, h,
                                                            gcol:gcol + ST]
                            t1 = rp.tile([128, ST], F32, tag="t1")
                            nc.vector.tensor_mul(
                                t1, acc, cos_sb[:, scol:scol + ST])
                            t2 = rp.tile([128, ST], F32, tag="t2")
                            nc.vector.tensor_mul(
                                t2[0:64], acc[64:128],
                                sin_sb[0:64, scol:scol + ST])
                            nc.vector.tensor_mul(
                                t2[64:128], acc[0:64],
                                sin_sb[64:128, scol:scol + ST])
                            nc.vector.tensor_sub(dst[0:64], t1[0:64],
                                                 t2[0:64])
                            nc.vector.tensor_add(dst[64:128], t1[64:128],
                                                 t2[64:128])
                        else:
                            # V: stage fp32, transpose 128x128 blocks to
                            # natural [s, hd] layout, store fp16.
                            vstage = stp.tile([128, ST], F16, tag="vst")
                            nc.scalar.copy(out=vstage, in_=acc)
                            for jj in range(ST // 128):
                                gc = gcol + jj * 128
                                b_idx, kblk = gc // S, (gc % S) // 128
                                pst = ptp.tile([128, 128], F16, tag="pt")
                                nc.tensor.transpose(
                                    pst, vstage[:, jj * 128:(jj + 1) * 128],
                                    ident)
                                nc.scalar.copy(
                                    out=Vhat[:, h, b_idx, kblk, 0:128],
                                    in_=pst)


def _attention_phase(nc, tc, cfg, QT, KT, Vhat, band_sb, ident, a2a_in):
    """Causal attention per (h, b); writes AO^T blocks into a2a_in slots."""
    B, S = cfg.B, cfg.S
    HPC, HD, ST, RPC = cfg.HPC, cfg.HD, cfg.ST, cfg.RPC

    with (
        tc.tile_pool(name="ptile", bufs=3) as ptp_,
        tc.tile_pool(name="attn_sm", bufs=8) as smp,
        tc.tile_pool(name="ao", bufs=4) as aop,
        tc.tile_pool(name="aot", bufs=4) as aotp,
        tc.tile_pool(name="psS", bufs=2, space="PSUM") as psS,
        tc.tile_pool(name="psO", bufs=4, space="PSUM") as psO,
        tc.tile_pool(name="psT", bufs=2, space="PSUM") as psT,
    ):
        for h in range(HPC):
            for b in range(B):
                for qt in range(S // ST):
                    q0 = qt * ST
                    nkb = (q0 + ST) // 128
                    po = [psO.tile([128, 130], F32, tag="po",
                                   name=f"po_{j}")
                          for j in range(ST // 128)]
                    for kb in range(nkb):
                        ps = psS.tile([128, ST], F32, tag="ps")
                        nc.tensor.matmul(
                            ps,
                            lhsT=KT[:, h, b * S + kb * 128:
                                    b * S + kb * 128 + 128],
                            rhs=QT[:, h, b * S + q0:
                                   b * S + q0 + ST],
                            start=True, stop=True)
                        d = kb * 128 - q0
                        if d >= 0:
                            nc.vector.tensor_add(
                                ps, ps,
                                band_sb[:, (ST - 128) - d:
                                        (2 * ST - 128) - d])
                        pt = ptp_.tile([128, ST], F16, tag="pt")
                        nc.scalar.activation(
                            pt, ps, mybir.ActivationFunctionType.Exp)
                        for j in range(ST // 128):
                            nc.tensor.matmul(
                                po[j],
                                lhsT=pt[:, j * 128:(j + 1) * 128],
                                rhs=Vhat[:, h, b, kb, 0:130],
                                start=(kb == 0),
                                stop=(kb == nkb - 1))
                    for j in range(ST // 128):
                        rec = smp.tile([128, 1], F32, tag="rec")
                        nc.vector.reciprocal(rec, po[j][:, 128:129])
                        ao = aop.tile([128, 128], F16, tag="ao")
                        nc.vector.tensor_scalar_mul(ao, po[j][:, 0:128], rec)
                        pst = psT.tile([128, 128], F16, tag="pst")
                        nc.tensor.transpose(pst, ao, ident)
                        aot = aotp.tile([128, 128], F16, tag="aot")
                        nc.scalar.copy(out=aot, in_=pst)
                        r0 = b * S + q0 + j * 128
                        slot, soff = r0 // RPC, r0 % RPC
                        nc.sync.dma_start(
                            out=a2a_in[slot, h * HD:(h + 1) * HD,
                                       soff:soff + 128],
                            in_=aot)


def _outproj_phase(nc, tc, cfg, a2a_out, wo, out_rows):
    """out_rows[s', :] = AO^T_full[:, s'].T @ wo  (contraction over 2048)."""
    D, HPC, HD, RPC, NHB = cfg.D, cfg.HPC, cfg.HD, cfg.RPC, cfg.NHB
    NDC = D // 512

    with (
        tc.tile_pool(name="wo_sb", bufs=1) as wop,
        tc.tile_pool(name="aot_sb", bufs=1) as aip,
        tc.tile_pool(name="ob", bufs=4) as obp,
        tc.tile_pool(name="psU", bufs=8, space="PSUM") as psU,
    ):
        wo_sb, aot_sb = [], []
        for i in range(NHB):
            w = wop.tile([128, D], F16, tag=f"wo{i}")
            nc.sync.dma_start(out=w, in_=wo[i * 128:(i + 1) * 128, :])
            wo_sb.append(w)
            a = aip.tile([128, RPC], F16, tag=f"ai{i}")
            nc.sync.dma_start(
                out=a, in_=a2a_out[i // HPC,
                                   (i % HPC) * HD:(i % HPC) * HD + 128, :])
            aot_sb.append(a)

        for ss in range(RPC // 128):
            pu = [psU.tile([128, 512], F32, tag="pu", name=f"pu_{dct}")
                  for dct in range(NDC)]
            for i in range(NHB):
                for dct in range(NDC):
                    nc.tensor.matmul(
                        pu[dct],
                        lhsT=aot_sb[i][:, ss * 128:(ss + 1) * 128],
                        rhs=wo_sb[i][:, dct * 512:(dct + 1) * 512],
                        start=(i == 0), stop=(i == NHB - 1))
            for dct in range(NDC):
                ob = obp.tile([128, 512], F32, tag="ob")
                nc.scalar.copy(out=ob, in_=pu[dct])
                nc.sync.dma_start(
                    out=out_rows[ss * 128:(ss + 1) * 128,
                                 dct * 512:(dct + 1) * 512],
                    in_=ob)


# ---------------------------------------------------------------------------
# Host side
# ---------------------------------------------------------------------------

def _rope_perm(hd):
    """head-dim permutation: (0,2,4,...,hd-2, 1,3,...,hd-1)."""
    return np.concatenate([np.arange(0, hd, 2), np.arange(1, hd, 2)])


def prepare_inputs(cfg: Cfg, x, freq_cis, wq_base, wk_base, wv_base, head_a,
                   head_b, q_a, q_b, k_a, k_b, v_a, v_b, wo):
    """Fold LoRA + softmax scale + RoPE permutation into per-core weights."""
    B, S, D, HD, HPC, NC_ = cfg.B, cfg.S, cfg.D, cfg.HD, cfg.HPC, cfg.NCORES
    HEADS = HPC * NC_
    LORA_SCALE = 2.0
    sm = 1.0 / math.sqrt(HD)

    def fold(w_base, oa, ob):
        w = w_base.astype(np.float64).copy()
        only = LORA_SCALE * (oa.astype(np.float64) @ ob.astype(np.float64))
        hoff = LORA_SCALE * (head_a.astype(np.float64)
                             @ head_b.astype(np.float64))
        w += hoff
        w += np.tile(only, (1, HEADS))
        return w

    wq_eff = fold(wq_base, q_a, q_b) * sm
    wk_eff = fold(wk_base, k_a, k_b)
    wv_eff = fold(wv_base, v_a, v_b)

    perm = _rope_perm(HD)
    for h in range(HEADS):
        cols = h * HD + perm
        wq_eff[:, h * HD:(h + 1) * HD] = wq_eff[:, cols]
        wk_eff[:, h * HD:(h + 1) * HD] = wk_eff[:, cols]
    wq_eff = wq_eff.astype(np.float16)
    wk_eff = wk_eff.astype(np.float16)
    wv_eff = wv_eff.astype(np.float16)

    xT = np.ascontiguousarray(x.reshape(cfg.BS, D).T.astype(np.float16))

    cos = freq_cis[:S, :, 0].T.astype(np.float32)   # [64, S]
    sin = freq_cis[:S, :, 1].T.astype(np.float32)
    cosT = np.ascontiguousarray(np.concatenate([cos, cos], axis=0))
    sinT = np.ascontiguousarray(np.concatenate([sin, sin], axis=0))

    ST = cfg.ST
    ii = np.arange(128)[:, None]
    cc = np.arange(2 * ST - 128)[None, :]
    band = np.where(ii <= cc - (ST - 128), 0.0, NEG).astype(np.float32)

    in_maps = []
    for c in range(NC_):
        sl = slice(c * HPC * HD, (c + 1) * HPC * HD)
        in_maps.append(dict(
            xT=xT,
            wq=np.ascontiguousarray(wq_eff[:, sl]),
            wk=np.ascontiguousarray(wk_eff[:, sl]),
            wv=np.ascontiguousarray(wv_eff[:, sl]),
            wo=np.ascontiguousarray(wo.astype(np.float16)),
            cosT=cosT, sinT=sinT, band=band,
        ))
    return in_maps


_BASS_CACHE = {}


def _get_bass(cfg: Cfg):
    key = (cfg.B, cfg.S, cfg.D, cfg.HPC, cfg.NCORES)
    if key not in _BASS_CACHE:
        _BASS_CACHE[key] = build_bass(cfg)
    return _BASS_CACHE[key]


def kernel(**inputs) -> np.ndarray:
    from concourse.bass_utils import run_bass_kernel_spmd

    x = np.asarray(inputs["x"])
    B, S, D = x.shape
    cfg = Cfg(B=B, S=S, D=D, HEADS=16, NCORES=8)
    in_maps = prepare_inputs(cfg, **{k: np.asarray(v)
                                     for k, v in inputs.items()})
    nc = _get_bass(cfg)
    res = run_bass_kernel_spmd(nc, in_maps, list(range(cfg.NCORES)))
    rows = np.concatenate([res.results[c]["out_rows"]
                           for c in range(cfg.NCORES)], axis=0)
    return rows.reshape(B, S, D).astype(np.float32)


# revision 13
# speedup vs baseline: 1.0054x; 1.0054x over previous
"""Trainium2 Bass kernel for LoRA-augmented causal attention.

Reference computation (per nn_Attention_31688268710508):
  x:(B,S,D) -> q/k/v = x@W* + broadcast LoRA + shared head-offset LoRA,
  RoPE(q,k), causal softmax attention per (b,head), out-proj with wo.

Strategy (8 NeuronCores, tensor-parallel over heads):
  * All rank-8 LoRA terms are rank-8 matrices in the weight space, so they are
    folded into effective projection weights on the host (x @ (A@B) == (x@A)@B
    up to fp rounding).  The softmax 1/sqrt(HD) scale is folded into Wq.
  * RoPE pairs (2i,2i+1) are moved to (i, i+64) by permuting Wq/Wk columns
    (scores are invariant to a shared permutation of q/k head dims), making
    RoPE a half-partition-block rotation in the on-chip [hd, s] layout.
  * Each core projects QT/KT/V for its 2 heads (W stationary, x^T streamed),
    runs causal attention with S^T = K^T.T @ Q^T blocks, exp -> fp16 P^T,
    and A@V via P^T-stationary matmuls against V augmented with a ones
    column (giving the softmax denominator for free).
  * Attention outputs are transposed on the PE and exchanged with an
    AllToAll so each core owns 512 sequence rows with all 2048 head dims,
    then multiplies with the full wo to produce its output row shard.
"""

import math
import os
import sys
from contextlib import ExitStack

import numpy as np

for _p in ("/opt/trn_rl_repo", "/root/.axon_site/_ro/trn_rl_repo"):
    if os.path.isdir(_p) and _p not in sys.path:
        sys.path.insert(0, _p)

import concourse.bass as bass  # noqa: E402
from concourse import bacc  # noqa: E402
import concourse.mybir as mybir  # noqa: E402
import concourse.tile as tile  # noqa: E402
from concourse.masks import make_identity  # noqa: E402

F32 = mybir.dt.float32
F32R = mybir.dt.float32r
F16 = mybir.dt.float16

NEG = -1.0e30


class Cfg:
    def __init__(self, B=2, S=2048, D=2048, HEADS=16, NCORES=8):
        self.B, self.S, self.D, self.NCORES = B, S, D, NCORES
        self.HD = 128
        self.HPC = HEADS // NCORES          # heads per core
        self.BS = B * S
        self.RPC = self.BS // NCORES        # output rows per core
        self.ST = 512                       # free-dim tile (q tile, s tile)
        self.SG = min(1024, self.BS)        # x^T slab width (s cols per slab)
        self.NDB = D // 128                 # d blocks (contraction)
        self.NSG = self.BS // self.SG
        self.NST_G = self.SG // self.ST     # s tiles per slab
        self.NQT = S // self.ST             # q tiles per (b, h)
        self.NKB = S // 128                 # k blocks per (b, h)
        self.NHB = (HEADS * self.HD) // 128  # head-dim blocks, total (out-proj K)
        assert self.HD == 128 and D % 512 == 0 and S % self.ST == 0
        assert self.RPC % 128 == 0 and self.SG % self.ST == 0


def build_bass(cfg: Cfg) -> bass.Bass:
    B, S, D = cfg.B, cfg.S, cfg.D
    HPC, HD, ST, SG = cfg.HPC, cfg.HD, cfg.ST, cfg.SG
    NC_, RPC = cfg.NCORES, cfg.RPC

    nc = bacc.Bacc("TRN2", target_bir_lowering=False, debug=False,
                   num_devices=NC_)

    xT = nc.declare_dram_parameter("xT", [D, cfg.BS], F16, isOutput=False)
    wq = nc.declare_dram_parameter("wq", [D, HPC * HD], F16, isOutput=False)
    wk = nc.declare_dram_parameter("wk", [D, HPC * HD], F16, isOutput=False)
    wv = nc.declare_dram_parameter("wv", [D, HPC * HD], F16, isOutput=False)
    wo = nc.declare_dram_parameter("wo", [NC_ * HPC * HD, D], F16,
                                   isOutput=False)
    cosT = nc.declare_dram_parameter("cosT", [128, S], F32, isOutput=False)
    sinT = nc.declare_dram_parameter("sinT", [128, S], F32, isOutput=False)
    band = nc.declare_dram_parameter("band", [128, 2 * ST - 128], F32,
                                     isOutput=False)
    out_rows = nc.declare_dram_parameter("out_rows", [RPC, D], F32,
                                         isOutput=True)

    a2a_in = nc.dram_tensor("a2a_in", [NC_, HPC * HD, RPC], F16)
    a2a_out = nc.dram_tensor("a2a_out", [NC_, HPC * HD, RPC], F16)

    with tile.TileContext(nc) as tc:
        with tc.tile_pool(name="const", bufs=1) as constp:
            ident = constp.tile([128, 128], F16)
            make_identity(nc, ident)
            band_sb = constp.tile([128, 2 * ST - 128], F32)
            nc.sync.dma_start(out=band_sb, in_=band[:, :])
            ones_sb = constp.tile([128, 128], F16)
            nc.vector.memset(ones_sb, 1.0)

            qkv_ctx = ExitStack()
            qtp = qkv_ctx.enter_context(tc.tile_pool(name="qt", bufs=1))
            ktp = qkv_ctx.enter_context(tc.tile_pool(name="kt", bufs=1))
            vhp = qkv_ctx.enter_context(tc.tile_pool(name="vhat", bufs=1))
            # pools close LIFO at TileContext exit; QKV stays resident to
            # the end (SBUF has room).
            QT = qtp.tile([128, HPC, cfg.BS], F16)
            KT = ktp.tile([128, HPC, cfg.BS], F16)
            Vhat = vhp.tile([128, HPC, B, S // 128, 128], F16)

            _projection_phase(nc, tc, cfg, xT, wq, wk, wv, cosT, sinT,
                              QT, KT, Vhat, ident)

            with (
                tc.tile_pool(name="wo_sb", bufs=1) as wop,
                tc.tile_pool(name="aot_sb", bufs=1) as aip,
            ):
                # wo preload overlaps the attention phase (slots become
                # free once the projection pools close).
                wo_sb = []
                for i in range(cfg.NHB):
                    w = wop.tile([128, D], F16, tag=f"wo{i}", name=f"wo_sb{i}")
                    nc.sync.dma_start(out=w, in_=wo[i * 128:(i + 1) * 128, :])
                    wo_sb.append(w)

                _attention_phase(nc, tc, cfg, QT, KT, Vhat, band_sb, ones_sb,
                                 a2a_in)

                nc.gpsimd.collective_compute(
                    "AllToAll",
                    mybir.AluOpType.bypass,
                    replica_groups=[list(range(NC_))],
                    ins=[a2a_in[:, :, :]],
                    outs=[a2a_out[:, :, :]],
                )

                _outproj_phase(nc, tc, cfg, a2a_out, wo_sb, aip, out_rows)
            qkv_ctx.close()

    nc.finalize()
    return nc


def _projection_phase(nc, tc, cfg, xT, wq, wk, wv, cosT, sinT, QT, KT, Vhat,
                      ident):
    """QT/KT (RoPE'd, [hd, s] layout) and Vhat ([s, hd]+ones, fp16)."""
    B, S = cfg.B, cfg.S
    HPC, ST, SG = cfg.HPC, cfg.ST, cfg.SG
    NDB, NSG, NST_G = cfg.NDB, cfg.NSG, cfg.NST_G
    w_drams = [wq, wk, wv]

    with (
        tc.tile_pool(name="xslab", bufs=NDB + 12) as xp,
        tc.tile_pool(name="wbuf", bufs=4) as wp,
        tc.tile_pool(name="tables", bufs=1) as tbp,
        tc.tile_pool(name="ropet", bufs=4) as rp,
        tc.tile_pool(name="vstage", bufs=4) as stp,
        tc.tile_pool(name="pacc", bufs=6, space="PSUM") as pap,
        tc.tile_pool(name="ptrans", bufs=2, space="PSUM") as ptp,
    ):
        cos_sb = tbp.tile([128, S], F32)
        nc.sync.dma_start(out=cos_sb, in_=cosT[:, :])
        sin_sb = tbp.tile([128, S], F32)
        nc.sync.dma_start(out=sin_sb, in_=sinT[:, :])

        for g in range(NSG):
            xs = []
            for db in range(NDB):
                t = xp.tile([128, SG], F16, tag="xs")
                nc.sync.dma_start(
                    out=t, in_=xT[db * 128:(db + 1) * 128,
                                  g * SG:(g + 1) * SG])
                xs.append(t)

            for proj in range(3):
                psums = [[pap.tile([128, ST], F32, tag="pacc",
                                   name=f"pacc_{h}_{st}")
                          for st in range(NST_G)] for h in range(HPC)]
                for db in range(NDB):
                    w_t = wp.tile([128, HPC * 128], F16, tag="w")
                    nc.sync.dma_start(
                        out=w_t, in_=w_drams[proj][db * 128:(db + 1) * 128, :])
                    for h in range(HPC):
                        for st in range(NST_G):
                            nc.tensor.matmul(
                                psums[h][st],
                                lhsT=w_t[:, h * 128:(h + 1) * 128],
                                rhs=xs[db][:, st * ST:(st + 1) * ST],
                                start=(db == 0),
                                stop=(db == NDB - 1),
                            )
                for h in range(HPC):
                    for st in range(NST_G):
                        gcol = g * SG + st * ST     # column in [0, B*S)
                        scol = gcol % S             # column in rope tables
                        acc = psums[h][st]
                        if proj < 2:
                            # RoPE: rows 0:64 = "real", 64:128 = "imag".
                            dst = (QT if proj == 0 else KT)[:, h,
                                                            gcol:gcol + ST]
                            t1 = rp.tile([128, ST], F32, tag="t1")
                            nc.vector.tensor_mul(
                                t1, acc, cos_sb[:, scol:scol + ST])
                            t2 = rp.tile([128, ST], F32, tag="t2")
                            nc.vector.tensor_mul(
                                t2[0:64], acc[64:128],
                                sin_sb[0:64, scol:scol + ST])
                            nc.vector.tensor_mul(
                                t2[64:128], acc[0:64],
                                sin_sb[64:128, scol:scol + ST])
                            nc.vector.tensor_sub(dst[0:64], t1[0:64],
                                                 t2[0:64])
                            nc.vector.tensor_add(dst[64:128], t1[64:128],
                                                 t2[64:128])
                        else:
                            # V: stage fp32, transpose 128x128 blocks to
                            # natural [s, hd] layout, store fp16.
                            vstage = stp.tile([128, ST], F16, tag="vst")
                            nc.scalar.copy(out=vstage, in_=acc)
                            for jj in range(ST // 128):
                                gc = gcol + jj * 128
                                b_idx, kblk = gc // S, (gc % S) // 128
                                pst = ptp.tile([128, 128], F16, tag="pt")
                                nc.tensor.transpose(
                                    pst, vstage[:, jj * 128:(jj + 1) * 128],
                                    ident)
                                nc.scalar.copy(
                                    out=Vhat[:, h, b_idx, kblk, :],
                                    in_=pst)


def _attention_phase(nc, tc, cfg, QT, KT, Vhat, band_sb, ones_sb, a2a_in):
    """Causal attention per (h, b); O^T blocks written into a2a_in slots.

    S^T[k,q] = K^T.T @ Q^T per 128-k block; exp -> fp16 P^T; O^T accumulates
    V-stationary matmuls (out [hd, q]); softmax denominator = ones^T @ sum_kb
    P^T (DVE accumulates P^T blocks, one 1-row matmul reduces partitions).
    """
    B, S = cfg.B, cfg.S
    HPC, HD, ST, RPC = cfg.HPC, cfg.HD, cfg.ST, cfg.RPC
    CSZ = min(ST, RPC)

    with (
        tc.tile_pool(name="ptile", bufs=4) as ptp_,
        tc.tile_pool(name="pacc", bufs=3) as pap,
        tc.tile_pool(name="aot", bufs=3) as aotp,
        tc.tile_pool(name="rec", bufs=4) as rcp,
        tc.tile_pool(name="psS", bufs=3, space="PSUM") as psS,
        tc.tile_pool(name="psO", bufs=2, space="PSUM") as psO,
        tc.tile_pool(name="psD", bufs=2, space="PSUM") as psD,
    ):
        for h in range(HPC):
            for b in range(B):
                for qt in range(S // ST):
                    q0 = qt * ST
                    nkb = (q0 + ST) // 128
                    po = psO.tile([128, ST], F32, tag="po")
                    pa = pap.tile([128, ST], F16, tag="pa")
                    for kb in range(nkb):
                        ps = psS.tile([128, ST], F32, tag="ps")
                        nc.tensor.matmul(
                            ps,
                            lhsT=KT[:, h, b * S + kb * 128:
                                    b * S + kb * 128 + 128],
                            rhs=QT[:, h, b * S + q0:
                                   b * S + q0 + ST],
                            start=True, stop=True)
                        d = kb * 128 - q0
                        if d >= 0:
                            nc.vector.tensor_add(
                                ps, ps,
                                band_sb[:, (ST - 128) - d:
                                        (2 * ST - 128) - d])
                        pt = ptp_.tile([128, ST], F16, tag="pt")
                        nc.scalar.activation(
                            pt, ps, mybir.ActivationFunctionType.Exp)
                        nc.tensor.matmul(
                            po,
                            lhsT=Vhat[:, h, b, kb, :],
                            rhs=pt,
                            start=(kb == 0),
                            stop=(kb == nkb - 1))
                        if kb == 0:
                            nc.vector.tensor_copy(pa, pt)
                        else:
                            nc.vector.tensor_add(pa, pa, pt)
                    pd = psD.tile([128, ST], F32, tag="pd")
                    nc.tensor.matmul(pd, lhsT=ones_sb, rhs=pa,
                                     start=True, stop=True)
                    rec = rcp.tile([128, ST], F32, tag="rec")
                    nc.vector.reciprocal(rec, pd)
                    aot = aotp.tile([128, ST], F16, tag="aot")
                    nc.vector.tensor_mul(aot, po, rec)
                    for ci in range(ST // CSZ):
                        r0 = b * S + q0 + ci * CSZ
                        nc.sync.dma_start(
                            out=a2a_in[r0 // RPC, h * HD:(h + 1) * HD,
                                       r0 % RPC:r0 % RPC + CSZ],
                            in_=aot[:, ci * CSZ:(ci + 1) * CSZ])


def _outproj_phase(nc, tc, cfg, a2a_out, wo_sb, aip, out_rows):
    """out_rows[s', :] = AO^T_full[:, s'].T @ wo  (contraction over 2048)."""
    D, HPC, HD, RPC, NHB = cfg.D, cfg.HPC, cfg.HD, cfg.RPC, cfg.NHB
    NDC = D // 512

    with (
        tc.tile_pool(name="ob", bufs=4) as obp,
        tc.tile_pool(name="psU", bufs=8, space="PSUM") as psU,
    ):
        aot_sb = []
        for i in range(NHB):
            a = aip.tile([128, RPC], F16, tag=f"ai{i}", name=f"ai_sb{i}")
            nc.sync.dma_start(
                out=a, in_=a2a_out[i // HPC,
                                   (i % HPC) * HD:(i % HPC) * HD + 128, :])
            aot_sb.append(a)

        for ss in range(RPC // 128):
            pu = [psU.tile([128, 512], F32, tag="pu", name=f"pu_{dct}")
                  for dct in range(NDC)]
            for i in range(NHB):
                for dct in range(NDC):
                    nc.tensor.matmul(
                        pu[dct],
                        lhsT=aot_sb[i][:, ss * 128:(ss + 1) * 128],
                        rhs=wo_sb[i][:, dct * 512:(dct + 1) * 512],
                        start=(i == 0), stop=(i == NHB - 1))
            for dct in range(NDC):
                ob = obp.tile([128, 512], F32, tag="ob")
                nc.scalar.copy(out=ob, in_=pu[dct])
                nc.sync.dma_start(
                    out=out_rows[ss * 128:(ss + 1) * 128,
                                 dct * 512:(dct + 1) * 512],
                    in_=ob)


# ---------------------------------------------------------------------------
# Host side
# ---------------------------------------------------------------------------

def _rope_perm(hd):
    """head-dim permutation: (0,2,4,...,hd-2, 1,3,...,hd-1)."""
    return np.concatenate([np.arange(0, hd, 2), np.arange(1, hd, 2)])


def prepare_inputs(cfg: Cfg, x, freq_cis, wq_base, wk_base, wv_base, head_a,
                   head_b, q_a, q_b, k_a, k_b, v_a, v_b, wo):
    """Fold LoRA + softmax scale + RoPE permutation into per-core weights."""
    B, S, D, HD, HPC, NC_ = cfg.B, cfg.S, cfg.D, cfg.HD, cfg.HPC, cfg.NCORES
    HEADS = HPC * NC_
    LORA_SCALE = 2.0
    sm = 1.0 / math.sqrt(HD)

    def fold(w_base, oa, ob):
        w = w_base.astype(np.float64).copy()
        only = LORA_SCALE * (oa.astype(np.float64) @ ob.astype(np.float64))
        hoff = LORA_SCALE * (head_a.astype(np.float64)
                             @ head_b.astype(np.float64))
        w += hoff
        w += np.tile(only, (1, HEADS))
        return w

    wq_eff = fold(wq_base, q_a, q_b) * sm
    wk_eff = fold(wk_base, k_a, k_b)
    wv_eff = fold(wv_base, v_a, v_b)

    perm = _rope_perm(HD)
    for h in range(HEADS):
        cols = h * HD + perm
        wq_eff[:, h * HD:(h + 1) * HD] = wq_eff[:, cols]
        wk_eff[:, h * HD:(h + 1) * HD] = wk_eff[:, cols]
    wq_eff = wq_eff.astype(np.float16)
    wk_eff = wk_eff.astype(np.float16)
    wv_eff = wv_eff.astype(np.float16)

    xT = np.ascontiguousarray(x.reshape(cfg.BS, D).T.astype(np.float16))

    cos = freq_cis[:S, :, 0].T.astype(np.float32)   # [64, S]
    sin = freq_cis[:S, :, 1].T.astype(np.float32)
    cosT = np.ascontiguousarray(np.concatenate([cos, cos], axis=0))
    sinT = np.ascontiguousarray(np.concatenate([sin, sin], axis=0))

    ST = cfg.ST
    ii = np.arange(128)[:, None]
    cc = np.arange(2 * ST - 128)[None, :]
    band = np.where(ii <= cc - (ST - 128), 0.0, NEG).astype(np.float32)

    in_maps = []
    for c in range(NC_):
        sl = slice(c * HPC * HD, (c + 1) * HPC * HD)
        in_maps.append(dict(
            xT=xT,
            wq=np.ascontiguousarray(wq_eff[:, sl]),
            wk=np.ascontiguousarray(wk_eff[:, sl]),
            wv=np.ascontiguousarray(wv_eff[:, sl]),
            wo=np.ascontiguousarray(wo.astype(np.float16)),
            cosT=cosT, sinT=sinT, band=band,
        ))
    return in_maps


_BASS_CACHE = {}


def _get_bass(cfg: Cfg):
    key = (cfg.B, cfg.S, cfg.D, cfg.HPC, cfg.NCORES)
    if key not in _BASS_CACHE:
        _BASS_CACHE[key] = build_bass(cfg)
    return _BASS_CACHE[key]


def kernel(**inputs) -> np.ndarray:
    from concourse.bass_utils import run_bass_kernel_spmd

    x = np.asarray(inputs["x"])
    B, S, D = x.shape
    cfg = Cfg(B=B, S=S, D=D, HEADS=16, NCORES=8)
    in_maps = prepare_inputs(cfg, **{k: np.asarray(v)
                                     for k, v in inputs.items()})
    nc = _get_bass(cfg)
    res = run_bass_kernel_spmd(nc, in_maps, list(range(cfg.NCORES)))
    rows = np.concatenate([res.results[c]["out_rows"]
                           for c in range(cfg.NCORES)], axis=0)
    return rows.reshape(B, S, D).astype(np.float32)


# revision 17
# speedup vs baseline: 1.0669x; 1.0612x over previous
"""Trainium2 Bass kernel for LoRA-augmented causal attention.

Reference computation (per nn_Attention_31688268710508):
  x:(B,S,D) -> q/k/v = x@W* + broadcast LoRA + shared head-offset LoRA,
  RoPE(q,k), causal softmax attention per (b,head), out-proj with wo.

Strategy (8 NeuronCores, tensor-parallel over heads):
  * All rank-8 LoRA terms are rank-8 matrices in the weight space, so they are
    folded into effective projection weights on the host (x @ (A@B) == (x@A)@B
    up to fp rounding).  The softmax 1/sqrt(HD) scale is folded into Wq.
  * RoPE pairs (2i,2i+1) are moved to (i, i+64) by permuting Wq/Wk columns
    (scores are invariant to a shared permutation of q/k head dims), making
    RoPE a half-partition-block rotation in the on-chip [hd, s] layout.
  * Each core projects QT/KT/V for its 2 heads (W stationary, x^T streamed),
    runs causal attention with S^T = K^T.T @ Q^T blocks, exp -> fp16 P^T,
    and A@V via P^T-stationary matmuls against V augmented with a ones
    column (giving the softmax denominator for free).
  * Attention outputs are transposed on the PE and exchanged with an
    AllToAll so each core owns 512 sequence rows with all 2048 head dims,
    then multiplies with the full wo to produce its output row shard.
"""

import math
import os
import sys
from contextlib import ExitStack

import numpy as np

for _p in ("/opt/trn_rl_repo", "/root/.axon_site/_ro/trn_rl_repo"):
    if os.path.isdir(_p) and _p not in sys.path:
        sys.path.insert(0, _p)

import concourse.bass as bass  # noqa: E402
from concourse import bacc  # noqa: E402
import concourse.mybir as mybir  # noqa: E402
import concourse.tile as tile  # noqa: E402
from concourse.masks import make_identity  # noqa: E402

F32 = mybir.dt.float32
F32R = mybir.dt.float32r
F16 = mybir.dt.float16

NEG = -1.0e30


class Cfg:
    def __init__(self, B=2, S=2048, D=2048, HEADS=16, NCORES=8):
        self.B, self.S, self.D, self.NCORES = B, S, D, NCORES
        self.HD = 128
        self.HPC = HEADS // NCORES          # heads per core
        self.BS = B * S
        self.RPC = self.BS // NCORES        # output rows per core
        self.ST = 512                       # free-dim tile (q tile, s tile)
        self.SG = min(1024, self.BS)        # x^T slab width (s cols per slab)
        self.NDB = D // 128                 # d blocks (contraction)
        self.NSG = self.BS // self.SG
        self.NST_G = self.SG // self.ST     # s tiles per slab
        self.NQT = S // self.ST             # q tiles per (b, h)
        self.NKB = S // 128                 # k blocks per (b, h)
        self.NHB = (HEADS * self.HD) // 128  # head-dim blocks, total (out-proj K)
        assert self.HD == 128 and D % 512 == 0 and S % self.ST == 0
        assert self.RPC % 128 == 0 and self.SG % self.ST == 0


def build_bass(cfg: Cfg) -> bass.Bass:
    B, S, D = cfg.B, cfg.S, cfg.D
    HPC, HD, ST, SG = cfg.HPC, cfg.HD, cfg.ST, cfg.SG
    NC_, RPC = cfg.NCORES, cfg.RPC

    nc = bacc.Bacc("TRN2", target_bir_lowering=False, debug=False,
                   num_devices=NC_)

    xT = nc.declare_dram_parameter("xT", [D, cfg.BS], F16, isOutput=False)
    wq = nc.declare_dram_parameter("wq", [D, HPC * HD], F16, isOutput=False)
    wk = nc.declare_dram_parameter("wk", [D, HPC * HD], F16, isOutput=False)
    wv = nc.declare_dram_parameter("wv", [D, HPC * HD], F16, isOutput=False)
    wo = nc.declare_dram_parameter("wo", [NC_ * HPC * HD, D], F16,
                                   isOutput=False)
    cosT = nc.declare_dram_parameter("cosT", [128, S], F32, isOutput=False)
    sinT = nc.declare_dram_parameter("sinT", [128, S], F32, isOutput=False)
    band = nc.declare_dram_parameter("band", [128, 2 * ST - 128], F32,
                                     isOutput=False)
    out_rows = nc.declare_dram_parameter("out_rows", [RPC, D], F32,
                                         isOutput=True)

    a2a_ins = [nc.dram_tensor(f"a2a_in{h}", [NC_, HD, RPC], F16)
               for h in range(HPC)]
    a2a_outs = [nc.dram_tensor(f"a2a_out{h}", [NC_, HD, RPC], F16)
                for h in range(HPC)]

    with tile.TileContext(nc) as tc:
        with tc.tile_pool(name="const", bufs=1) as constp:
            ident = constp.tile([128, 128], F16)
            make_identity(nc, ident)
            band_sb = constp.tile([128, 2 * ST - 128], F32)
            nc.sync.dma_start(out=band_sb, in_=band[:, :])
            ones_sb = constp.tile([128, 128], F16)
            nc.vector.memset(ones_sb, 1.0)

            qkv_ctx = ExitStack()
            qtp = qkv_ctx.enter_context(tc.tile_pool(name="qt", bufs=1))
            ktp = qkv_ctx.enter_context(tc.tile_pool(name="kt", bufs=1))
            vhp = qkv_ctx.enter_context(tc.tile_pool(name="vhat", bufs=1))
            # pools close LIFO at TileContext exit; QKV stays resident to
            # the end (SBUF has room).
            QT = qtp.tile([128, HPC, cfg.BS], F16)
            KT = ktp.tile([128, HPC, cfg.BS], F16)
            Vhat = vhp.tile([128, HPC, B, S // 128, 128], F16)

            _projection_phase(nc, tc, cfg, xT, wq, wk, wv, cosT, sinT,
                              QT, KT, Vhat, ident)

            with (
                tc.tile_pool(name="wo_sb", bufs=1) as wop,
                tc.tile_pool(name="aot_sb", bufs=1) as aip,
            ):
                # wo preload overlaps the attention phase (slots become
                # free once the projection pools close).
                wo_sb = []
                for i in range(cfg.NHB):
                    w = wop.tile([128, D], F16, tag=f"wo{i}", name=f"wo_sb{i}")
                    nc.sync.dma_start(out=w, in_=wo[i * 128:(i + 1) * 128, :])
                    wo_sb.append(w)

                _attention_phase(nc, tc, cfg, QT, KT, Vhat, band_sb, ones_sb,
                                 a2a_ins, a2a_outs)

                _outproj_phase(nc, tc, cfg, a2a_outs, wo_sb, aip, out_rows)
            qkv_ctx.close()

    nc.finalize()
    return nc


def _projection_phase(nc, tc, cfg, xT, wq, wk, wv, cosT, sinT, QT, KT, Vhat,
                      ident):
    """QT/KT (RoPE'd, [hd, s] layout) and Vhat ([s, hd]+ones, fp16)."""
    B, S = cfg.B, cfg.S
    HPC, ST, SG = cfg.HPC, cfg.ST, cfg.SG
    NDB, NSG, NST_G = cfg.NDB, cfg.NSG, cfg.NST_G
    w_drams = [wq, wk, wv]

    with (
        tc.tile_pool(name="xslab", bufs=NDB + 12) as xp,
        tc.tile_pool(name="wbuf", bufs=4) as wp,
        tc.tile_pool(name="tables", bufs=1) as tbp,
        tc.tile_pool(name="ropet", bufs=4) as rp,
        tc.tile_pool(name="vstage", bufs=4) as stp,
        tc.tile_pool(name="pacc", bufs=6, space="PSUM") as pap,
        tc.tile_pool(name="ptrans", bufs=2, space="PSUM") as ptp,
    ):
        cos_sb = tbp.tile([128, S], F32)
        sin_sb = tbp.tile([128, S], F32)

        for g in range(NSG):
            xs = []
            for db in range(NDB):
                t = xp.tile([128, SG], F16, tag="xs")
                nc.sync.dma_start(
                    out=t, in_=xT[db * 128:(db + 1) * 128,
                                  g * SG:(g + 1) * SG])
                xs.append(t)
            if g == 0:
                # tables are first needed at the first RoPE drain; issuing
                # after the first slab keeps the first matmul off the
                # table-DMA tail.
                nc.sync.dma_start(out=cos_sb, in_=cosT[:, :])
                nc.sync.dma_start(out=sin_sb, in_=sinT[:, :])

            for proj in range(3):
                psums = [[pap.tile([128, ST], F32, tag="pacc",
                                   name=f"pacc_{h}_{st}")
                          for st in range(NST_G)] for h in range(HPC)]
                for db in range(NDB):
                    w_t = wp.tile([128, HPC * 128], F16, tag="w")
                    nc.sync.dma_start(
                        out=w_t, in_=w_drams[proj][db * 128:(db + 1) * 128, :])
                    for h in range(HPC):
                        for st in range(NST_G):
                            nc.tensor.matmul(
                                psums[h][st],
                                lhsT=w_t[:, h * 128:(h + 1) * 128],
                                rhs=xs[db][:, st * ST:(st + 1) * ST],
                                start=(db == 0),
                                stop=(db == NDB - 1),
                            )
                for h in range(HPC):
                    for st in range(NST_G):
                        gcol = g * SG + st * ST     # column in [0, B*S)
                        scol = gcol % S             # column in rope tables
                        acc = psums[h][st]
                        if proj < 2:
                            # RoPE: rows 0:64 = "real", 64:128 = "imag".
                            dst = (QT if proj == 0 else KT)[:, h,
                                                            gcol:gcol + ST]
                            t1 = rp.tile([128, ST], F32, tag="t1")
                            nc.vector.tensor_mul(
                                t1, acc, cos_sb[:, scol:scol + ST])
                            t2 = rp.tile([128, ST], F32, tag="t2")
                            nc.vector.tensor_mul(
                                t2[0:64], acc[64:128],
                                sin_sb[0:64, scol:scol + ST])
                            nc.vector.tensor_mul(
                                t2[64:128], acc[0:64],
                                sin_sb[64:128, scol:scol + ST])
                            nc.vector.tensor_sub(dst[0:64], t1[0:64],
                                                 t2[0:64])
                            nc.vector.tensor_add(dst[64:128], t1[64:128],
                                                 t2[64:128])
                        else:
                            # V: stage fp32, transpose 128x128 blocks to
                            # natural [s, hd] layout, store fp16.
                            vstage = stp.tile([128, ST], F16, tag="vst")
                            nc.scalar.copy(out=vstage, in_=acc)
                            for jj in range(ST // 128):
                                gc = gcol + jj * 128
                                b_idx, kblk = gc // S, (gc % S) // 128
                                pst = ptp.tile([128, 128], F16, tag="pt")
                                nc.tensor.transpose(
                                    pst, vstage[:, jj * 128:(jj + 1) * 128],
                                    ident)
                                nc.scalar.copy(
                                    out=Vhat[:, h, b_idx, kblk, :],
                                    in_=pst)


def _attention_phase(nc, tc, cfg, QT, KT, Vhat, band_sb, ones_sb,
                     a2a_ins, a2a_outs):
    """Causal attention per (h, b); O^T blocks written into a2a_in slots.

    S^T[k,q] = K^T.T @ Q^T per 128-k block; exp -> fp16 P^T; O^T accumulates
    V-stationary matmuls (out [hd, q]); softmax denominator = ones^T @ sum_kb
    P^T (DVE accumulates P^T blocks, one 1-row matmul reduces partitions).
    """
    B, S = cfg.B, cfg.S
    HPC, HD, ST, RPC = cfg.HPC, cfg.HD, cfg.ST, cfg.RPC
    CSZ = min(ST, RPC)
    KPG = 2                       # k-blocks per exp group
    PT_W = KPG * ST

    with (
        tc.tile_pool(name="ptile", bufs=3) as ptp_,
        tc.tile_pool(name="pacc", bufs=2) as pap,
        tc.tile_pool(name="aot", bufs=3) as aotp,
        tc.tile_pool(name="rec", bufs=2) as rcp,
        tc.tile_pool(name="psS", bufs=2, space="PSUM") as psS,
        tc.tile_pool(name="psO", bufs=2, space="PSUM") as psO,
        tc.tile_pool(name="psD", bufs=2, space="PSUM") as psD,
    ):
        for h in range(HPC):
            for b in range(B):
                for qt in range(S // ST):
                    q0 = qt * ST
                    nkb = (q0 + ST) // 128
                    po = psO.tile([128, ST], F32, tag="po")
                    pa = pap.tile([128, PT_W], F16, tag="pa")
                    for kg in range(nkb // KPG):
                        ps = psS.tile([128, KPG * ST], F32, tag="ps")
                        for kk in range(KPG):
                            kb = kg * KPG + kk
                            nc.tensor.matmul(
                                ps[:, kk * ST:(kk + 1) * ST],
                                lhsT=KT[:, h, b * S + kb * 128:
                                        b * S + kb * 128 + 128],
                                rhs=QT[:, h, b * S + q0:
                                       b * S + q0 + ST],
                                start=True, stop=True)
                            d = kb * 128 - q0
                            if d >= 0:
                                nc.vector.tensor_add(
                                    ps[:, kk * ST:(kk + 1) * ST],
                                    ps[:, kk * ST:(kk + 1) * ST],
                                    band_sb[:, (ST - 128) - d:
                                            (2 * ST - 128) - d])
                        pt = ptp_.tile([128, KPG * ST], F16, tag="pt")
                        nc.scalar.activation(
                            pt, ps, mybir.ActivationFunctionType.Exp)
                        for kk in range(KPG):
                            kb = kg * KPG + kk
                            nc.tensor.matmul(
                                po,
                                lhsT=Vhat[:, h, b, kb, :],
                                rhs=pt[:, kk * ST:(kk + 1) * ST],
                                start=(kb == 0),
                                stop=(kb == nkb - 1))
                        if kg == 0:
                            nc.vector.tensor_copy(pa, pt)
                        else:
                            nc.vector.tensor_add(pa, pa, pt)
                    pd = psD.tile([128, ST], F32, tag="pd")
                    for ph in range(PT_W // ST):
                        nc.tensor.matmul(pd, lhsT=ones_sb,
                                         rhs=pa[:, ph * ST:(ph + 1) * ST],
                                         start=(ph == 0),
                                         stop=(ph == PT_W // ST - 1))
                    rec = rcp.tile([128, ST], F32, tag="rec")
                    nc.vector.reciprocal_approx_fast(out=rec, in_=pd)
                    aot = aotp.tile([128, ST], F16, tag="aot")
                    nc.vector.tensor_mul(aot, po, rec)
                    for ci in range(ST // CSZ):
                        r0 = b * S + q0 + ci * CSZ
                        nc.sync.dma_start(
                            out=a2a_ins[h][r0 // RPC, :,
                                           r0 % RPC:r0 % RPC + CSZ],
                            in_=aot[:, ci * CSZ:(ci + 1) * CSZ])
            nc.gpsimd.collective_compute(
                "AllToAll",
                mybir.AluOpType.bypass,
                replica_groups=[list(range(cfg.NCORES))],
                ins=[a2a_ins[h][:, :, :]],
                outs=[a2a_outs[h][:, :, :]],
            )


def _outproj_phase(nc, tc, cfg, a2a_outs, wo_sb, aip, out_rows):
    """out_rows[s', :] = AO^T_full[:, s'].T @ wo  (contraction over 2048)."""
    D, HPC, HD, RPC, NHB = cfg.D, cfg.HPC, cfg.HD, cfg.RPC, cfg.NHB
    NDC = D // 512

    with (
        tc.tile_pool(name="ob", bufs=4) as obp,
        tc.tile_pool(name="psU", bufs=8, space="PSUM") as psU,
    ):
        aot_sb = []
        for i in range(NHB):
            a = aip.tile([128, RPC], F16, tag=f"ai{i}", name=f"ai_sb{i}")
            nc.sync.dma_start(out=a, in_=a2a_outs[i % HPC][i // HPC, :, :])
            aot_sb.append(a)

        for ss in range(RPC // 128):
            pu = [psU.tile([128, 512], F32, tag="pu", name=f"pu_{dct}")
                  for dct in range(NDC)]
            for i in range(NHB):
                for dct in range(NDC):
                    nc.tensor.matmul(
                        pu[dct],
                        lhsT=aot_sb[i][:, ss * 128:(ss + 1) * 128],
                        rhs=wo_sb[i][:, dct * 512:(dct + 1) * 512],
                        start=(i == 0), stop=(i == NHB - 1))
            for dct in range(NDC):
                ob = obp.tile([128, 512], F32, tag="ob")
                nc.scalar.copy(out=ob, in_=pu[dct])
                nc.sync.dma_start(
                    out=out_rows[ss * 128:(ss + 1) * 128,
                                 dct * 512:(dct + 1) * 512],
                    in_=ob)


# ---------------------------------------------------------------------------
# Host side
# ---------------------------------------------------------------------------

def _rope_perm(hd):
    """head-dim permutation: (0,2,4,...,hd-2, 1,3,...,hd-1)."""
    return np.concatenate([np.arange(0, hd, 2), np.arange(1, hd, 2)])


def prepare_inputs(cfg: Cfg, x, freq_cis, wq_base, wk_base, wv_base, head_a,
                   head_b, q_a, q_b, k_a, k_b, v_a, v_b, wo):
    """Fold LoRA + softmax scale + RoPE permutation into per-core weights."""
    B, S, D, HD, HPC, NC_ = cfg.B, cfg.S, cfg.D, cfg.HD, cfg.HPC, cfg.NCORES
    HEADS = HPC * NC_
    LORA_SCALE = 2.0
    sm = 1.0 / math.sqrt(HD)

    def fold(w_base, oa, ob):
        w = w_base.astype(np.float64).copy()
        only = LORA_SCALE * (oa.astype(np.float64) @ ob.astype(np.float64))
        hoff = LORA_SCALE * (head_a.astype(np.float64)
                             @ head_b.astype(np.float64))
        w += hoff
        w += np.tile(only, (1, HEADS))
        return w

    wq_eff = fold(wq_base, q_a, q_b) * sm
    wk_eff = fold(wk_base, k_a, k_b)
    wv_eff = fold(wv_base, v_a, v_b)

    perm = _rope_perm(HD)
    for h in range(HEADS):
        cols = h * HD + perm
        wq_eff[:, h * HD:(h + 1) * HD] = wq_eff[:, cols]
        wk_eff[:, h * HD:(h + 1) * HD] = wk_eff[:, cols]
    wq_eff = wq_eff.astype(np.float16)
    wk_eff = wk_eff.astype(np.float16)
    wv_eff = wv_eff.astype(np.float16)

    xT = np.ascontiguousarray(x.reshape(cfg.BS, D).T.astype(np.float16))

    cos = freq_cis[:S, :, 0].T.astype(np.float32)   # [64, S]
    sin = freq_cis[:S, :, 1].T.astype(np.float32)
    cosT = np.ascontiguousarray(np.concatenate([cos, cos], axis=0))
    sinT = np.ascontiguousarray(np.concatenate([sin, sin], axis=0))

    ST = cfg.ST
    ii = np.arange(128)[:, None]
    cc = np.arange(2 * ST - 128)[None, :]
    band = np.where(ii <= cc - (ST - 128), 0.0, NEG).astype(np.float32)

    in_maps = []
    for c in range(NC_):
        sl = slice(c * HPC * HD, (c + 1) * HPC * HD)
        in_maps.append(dict(
            xT=xT,
            wq=np.ascontiguousarray(wq_eff[:, sl]),
            wk=np.ascontiguousarray(wk_eff[:, sl]),
            wv=np.ascontiguousarray(wv_eff[:, sl]),
            wo=np.ascontiguousarray(wo.astype(np.float16)),
            cosT=cosT, sinT=sinT, band=band,
        ))
    return in_maps


_BASS_CACHE = {}


def _get_bass(cfg: Cfg):
    key = (cfg.B, cfg.S, cfg.D, cfg.HPC, cfg.NCORES)
    if key not in _BASS_CACHE:
        _BASS_CACHE[key] = build_bass(cfg)
    return _BASS_CACHE[key]


def kernel(**inputs) -> np.ndarray:
    from concourse.bass_utils import run_bass_kernel_spmd

    x = np.asarray(inputs["x"])
    B, S, D = x.shape
    cfg = Cfg(B=B, S=S, D=D, HEADS=16, NCORES=8)
    in_maps = prepare_inputs(cfg, **{k: np.asarray(v)
                                     for k, v in inputs.items()})
    nc = _get_bass(cfg)
    res = run_bass_kernel_spmd(nc, in_maps, list(range(cfg.NCORES)))
    rows = np.concatenate([res.results[c]["out_rows"]
                           for c in range(cfg.NCORES)], axis=0)
    return rows.reshape(B, S, D).astype(np.float32)


# revision 18
# speedup vs baseline: 1.1911x; 1.1164x over previous
"""Trainium2 Bass kernel for LoRA-augmented causal attention.

Reference computation (per nn_Attention_31688268710508):
  x:(B,S,D) -> q/k/v = x@W* + broadcast LoRA + shared head-offset LoRA,
  RoPE(q,k), causal softmax attention per (b,head), out-proj with wo.

Strategy (8 NeuronCores, tensor-parallel over heads):
  * All rank-8 LoRA terms are folded into effective projection weights on
    the host (x @ (A@B) == (x@A)@B up to fp rounding).  The softmax
    1/sqrt(HD) scale is folded into Wq.
  * RoPE pairs (2i,2i+1) are moved to (i, i+64) by permuting Wq/Wk columns
    (scores are invariant to a shared q/k head-dim permutation), making
    RoPE a half-partition-block rotation in the on-chip [hd, s] layout.
  * fp16 operands everywhere on the PE (fp32 PSUM accumulation).
  * Projections (W stationary, x^T streamed) and causal attention
    (S^T = K^T.T Q^T blocks -> exp -> fp16 P^T -> V-stationary O^T
    matmuls; softmax denominator via DVE-accumulated P^T + one ones-matmul)
    are emitted INTERLEAVED so projection matmuls for (b,h) segment k+1
    hide the exp latency of attention segment k and keep the PE dense/warm.
  * Per-head AllToAll redistributes O^T by sequence rows; the first
    exchange overlaps the second head's attention.  Each core then
    multiplies with full wo for its 512-row output shard.
"""

import math
import os
import sys
from contextlib import ExitStack

import numpy as np

for _p in ("/opt/trn_rl_repo", "/root/.axon_site/_ro/trn_rl_repo"):
    if os.path.isdir(_p) and _p not in sys.path:
        sys.path.insert(0, _p)

import concourse.bass as bass  # noqa: E402
import concourse.mybir as mybir  # noqa: E402
import concourse.tile as tile  # noqa: E402
from concourse import bacc  # noqa: E402
from concourse.masks import make_identity  # noqa: E402

F32 = mybir.dt.float32
F16 = mybir.dt.float16
EXP = mybir.ActivationFunctionType.Exp

NEG = -1.0e30


class Cfg:
    def __init__(self, B=2, S=2048, D=2048, HEADS=16, NCORES=8):
        self.B, self.S, self.D, self.NCORES = B, S, D, NCORES
        self.HD = 128
        self.HPC = HEADS // NCORES          # heads per core
        self.BS = B * S
        self.RPC = self.BS // NCORES        # output rows per core
        self.ST = 512                       # free-dim tile (q tile, s tile)
        self.SG = min(1024, self.S)         # x^T slab width (s cols)
        self.NDB = D // 128                 # contraction blocks
        self.GPB = self.S // self.SG        # slabs per batch b
        self.NST_G = self.SG // self.ST     # s tiles per slab
        self.KPG = 2                        # k-blocks per exp group
        self.NHB = (HEADS * self.HD) // 128  # out-proj contraction blocks
        assert self.HD == 128 and D % 512 == 0 and S % self.ST == 0
        assert self.RPC % 128 == 0 and self.SG % self.ST == 0


def build_bass(cfg: Cfg) -> bass.Bass:
    B, S, D = cfg.B, cfg.S, cfg.D
    HPC, HD, ST, SG = cfg.HPC, cfg.HD, cfg.ST, cfg.SG
    NC_, RPC = cfg.NCORES, cfg.RPC

    nc = bacc.Bacc("TRN2", target_bir_lowering=False, debug=False,
                   num_devices=NC_)

    xT = nc.declare_dram_parameter("xT", [D, cfg.BS], F16, isOutput=False)
    wq = nc.declare_dram_parameter("wq", [D, HPC * HD], F16, isOutput=False)
    wk = nc.declare_dram_parameter("wk", [D, HPC * HD], F16, isOutput=False)
    wv = nc.declare_dram_parameter("wv", [D, HPC * HD], F16, isOutput=False)
    wo = nc.declare_dram_parameter("wo", [NC_ * HPC * HD, D], F16,
                                   isOutput=False)
    cosT = nc.declare_dram_parameter("cosT", [128, S], F32, isOutput=False)
    sinT = nc.declare_dram_parameter("sinT", [128, S], F32, isOutput=False)
    band = nc.declare_dram_parameter("band", [128, 2 * ST - 128], F32,
                                     isOutput=False)
    out_rows = nc.declare_dram_parameter("out_rows", [RPC, D], F32,
                                         isOutput=True)

    a2a_ins = [nc.dram_tensor(f"a2a_in{h}", [NC_, HD, RPC], F16)
               for h in range(HPC)]
    a2a_outs = [nc.dram_tensor(f"a2a_out{h}", [NC_, HD, RPC], F16)
                for h in range(HPC)]

    with tile.TileContext(nc) as tc:
        with ExitStack() as ctx:
            constp = ctx.enter_context(tc.tile_pool(name="const", bufs=1))
            ident = constp.tile([128, 128], F16)
            make_identity(nc, ident)
            band_sb = constp.tile([128, 2 * ST - 128], F32)
            ones_sb = constp.tile([128, 128], F16)
            nc.vector.memset(ones_sb, 1.0)

            qtp = ctx.enter_context(tc.tile_pool(name="qt", bufs=1))
            ktp = ctx.enter_context(tc.tile_pool(name="kt", bufs=1))
            vhp = ctx.enter_context(tc.tile_pool(name="vhat", bufs=1))
            QT = qtp.tile([128, HPC, cfg.BS], F16)
            KT = ktp.tile([128, HPC, cfg.BS], F16)
            Vhat = vhp.tile([128, HPC, B, S // 128, 128], F16)

            st = _State(nc, tc, cfg, xT, (wq, wk, wv), cosT, sinT, band,
                        band_sb, ones_sb, ident, QT, KT, Vhat, a2a_ins,
                        a2a_outs)
            with ExitStack() as phase_ctx:
                st.open_phase_pools(phase_ctx)

                segs = [(b, h) for b in range(B) for h in range(HPC)]
                # lag-1 interleave: attention segment k runs against
                # projection segment k+1's matmuls.
                pgens = [st.proj_segment(b, h) for (b, h) in segs]
                agens = [st.attn_segment(b, h) for (b, h) in segs]
                for _ in pgens[0]:
                    pass
                for k, ag in enumerate(agens):
                    pg = pgens[k + 1] if k + 1 < len(segs) else None
                    for _ in ag:
                        if pg is not None:
                            next(pg, None)
                    if pg is not None:
                        for _ in pg:
                            pass
                    b, h = segs[k]
                    if b == B - 1:
                        nc.gpsimd.collective_compute(
                            "AllToAll",
                            mybir.AluOpType.bypass,
                            replica_groups=[list(range(NC_))],
                            ins=[a2a_ins[h][:, :, :]],
                            outs=[a2a_outs[h][:, :, :]],
                        )

            with (
                tc.tile_pool(name="wo_sb", bufs=1) as wop,
                tc.tile_pool(name="aot_sb", bufs=1) as aip,
                tc.tile_pool(name="ob", bufs=4) as obp,
                tc.tile_pool(name="psU", bufs=8, space="PSUM") as psU,
            ):
                _outproj(nc, cfg, wo, a2a_outs, wop, aip, obp, psU, out_rows)

    nc.finalize()
    return nc


class _State:
    """Shared emission state for the interleaved proj/attention phases."""

    def __init__(self, nc, tc, cfg, xT, w_drams, cosT, sinT, band, band_sb,
                 ones_sb, ident, QT, KT, Vhat, a2a_ins, a2a_outs):
        self.nc, self.tc, self.cfg = nc, tc, cfg
        self.xT, self.w_drams = xT, w_drams
        self.cosT, self.sinT, self.band = cosT, sinT, band
        self.band_sb, self.ones_sb, self.ident = band_sb, ones_sb, ident
        self.QT, self.KT, self.Vhat = QT, KT, Vhat
        self.a2a_ins, self.a2a_outs = a2a_ins, a2a_outs
        self.slabs = {}          # g -> slab tile
        self.tables_loaded = False

    def open_phase_pools(self, ctx):
        tc = self.tc
        self.xp = ctx.enter_context(tc.tile_pool(name="xslab", bufs=2))
        self.wp = ctx.enter_context(tc.tile_pool(name="wpanel", bufs=3))
        self.tbp = ctx.enter_context(tc.tile_pool(name="tables", bufs=1))
        self.rp = ctx.enter_context(tc.tile_pool(name="ropet", bufs=4))
        self.stp = ctx.enter_context(tc.tile_pool(name="vstage", bufs=3))
        self.ptp = ctx.enter_context(tc.tile_pool(name="ptile", bufs=3))
        self.pap = ctx.enter_context(tc.tile_pool(name="pacc", bufs=2))
        self.aotp = ctx.enter_context(tc.tile_pool(name="aot", bufs=3))
        self.rcp = ctx.enter_context(tc.tile_pool(name="rec", bufs=2))
        self.psS = ctx.enter_context(
            tc.tile_pool(name="psS", bufs=2, space="PSUM"))
        self.psJ = ctx.enter_context(
            tc.tile_pool(name="psJ", bufs=2, space="PSUM"))
        self.psO = ctx.enter_context(
            tc.tile_pool(name="psO", bufs=1, space="PSUM"))
        self.psX = ctx.enter_context(
            tc.tile_pool(name="psX", bufs=1, space="PSUM"))
        self.cos_sb = self.tbp.tile([128, self.cfg.S], F32)
        self.sin_sb = self.tbp.tile([128, self.cfg.S], F32)

    def _slab(self, g):
        """Whole-slab x^T tile [128, NDB, SG], single batched DMA."""
        nc, cfg = self.nc, self.cfg
        if g not in self.slabs:
            t = self.xp.tile([128, cfg.NDB, cfg.SG], F16, tag="xs",
                             name=f"xs{g}")
            src = self.xT[:, g * cfg.SG:(g + 1) * cfg.SG].rearrange(
                "(db p) c -> p db c", p=128)
            nc.sync.dma_start(out=t, in_=src)
            self.slabs[g] = t
            if not self.tables_loaded:
                self.tables_loaded = True
                nc.sync.dma_start(out=self.band_sb, in_=self.band[:, :])
                nc.sync.dma_start(out=self.cos_sb, in_=self.cosT[:, :])
                nc.sync.dma_start(out=self.sin_sb, in_=self.sinT[:, :])
        return self.slabs[g]

    def proj_segment(self, b, h):
        """Generator: projections (Q,K,V) of head h over batch b's slabs.

        Yields after each contraction step (NST_G matmuls) so the driver
        can interleave attention work.
        """
        nc, cfg = self.nc, self.cfg
        S, SG, ST, NDB = cfg.S, cfg.SG, cfg.ST, cfg.NDB
        for g in range(b * cfg.GPB, (b + 1) * cfg.GPB):
            xs = self._slab(g)
            for proj in range(3):
                w_t = self.wp.tile([128, NDB, 128], F16, tag="w")
                src = self.w_drams[proj][:, h * 128:(h + 1) * 128].rearrange(
                    "(db p) c -> p db c", p=128)
                nc.sync.dma_start(out=w_t, in_=src)
                accs = [self.psJ.tile([128, ST], F32, tag="pj",
                                      name=f"pj{st_i}")
                        for st_i in range(cfg.NST_G)]
                for db in range(NDB):
                    for st_i in range(cfg.NST_G):
                        nc.tensor.matmul(
                            accs[st_i],
                            lhsT=w_t[:, db, :],
                            rhs=xs[:, db, st_i * ST:(st_i + 1) * ST],
                            start=(db == 0), stop=(db == NDB - 1))
                    yield
                for st_i in range(cfg.NST_G):
                    gcol = g * SG + st_i * ST
                    scol = gcol % S
                    acc = accs[st_i]
                    if proj < 2:
                        dst = (self.QT if proj == 0 else
                               self.KT)[:, h, gcol:gcol + ST]
                        t1 = self.rp.tile([128, ST], F32, tag="t1")
                        nc.vector.tensor_mul(
                            t1, acc, self.cos_sb[:, scol:scol + ST])
                        t2 = self.rp.tile([128, ST], F32, tag="t2")
                        nc.vector.tensor_mul(
                            t2[0:64], acc[64:128],
                            self.sin_sb[0:64, scol:scol + ST])
                        nc.vector.tensor_mul(
                            t2[64:128], acc[0:64],
                            self.sin_sb[64:128, scol:scol + ST])
                        nc.vector.tensor_sub(dst[0:64], t1[0:64], t2[0:64])
                        nc.vector.tensor_add(dst[64:128], t1[64:128],
                                             t2[64:128])
                    else:
                        vstage = self.stp.tile([128, ST], F16, tag="vst")
                        nc.scalar.copy(out=vstage, in_=acc)
                        for jj in range(ST // 128):
                            gc = gcol + jj * 128
                            b_idx, kblk = gc // S, (gc % S) // 128
                            pst = self.psX.tile([128, 128], F16, tag="px",
                                                name="pst")
                            nc.tensor.transpose(
                                pst, vstage[:, jj * 128:(jj + 1) * 128],
                                self.ident)
                            nc.scalar.copy(
                                out=self.Vhat[:, h, b_idx, kblk, :],
                                in_=pst)
                    yield

    def attn_segment(self, b, h):
        """Generator: causal attention for (h, b); yields per exp-group."""
        nc, cfg = self.nc, self.cfg
        S, ST, RPC = cfg.S, cfg.ST, cfg.RPC
        KPG = cfg.KPG
        CSZ = min(ST, RPC)
        for qt in range(S // ST):
            q0 = qt * ST
            nkb = (q0 + ST) // 128
            po = self.psO.tile([128, ST], F32, tag="po")
            pa = self.pap.tile([128, KPG * ST], F16, tag="pa")
            for kg in range(nkb // KPG):
                ps = self.psS.tile([128, KPG * ST], F32, tag="ps")
                for kk in range(KPG):
                    kb = kg * KPG + kk
                    nc.tensor.matmul(
                        ps[:, kk * ST:(kk + 1) * ST],
                        lhsT=self.KT[:, h, b * S + kb * 128:
                                     b * S + kb * 128 + 128],
                        rhs=self.QT[:, h, b * S + q0:b * S + q0 + ST],
                        start=True, stop=True)
                    d = kb * 128 - q0
                    if d >= 0:
                        nc.vector.tensor_add(
                            ps[:, kk * ST:(kk + 1) * ST],
                            ps[:, kk * ST:(kk + 1) * ST],
                            self.band_sb[:, (ST - 128) - d:
                                         (2 * ST - 128) - d])
                pt = self.ptp.tile([128, KPG * ST], F16, tag="pt")
                nc.scalar.activation(pt, ps, EXP)
                for kk in range(KPG):
                    kb = kg * KPG + kk
                    nc.tensor.matmul(
                        po,
                        lhsT=self.Vhat[:, h, b, kb, :],
                        rhs=pt[:, kk * ST:(kk + 1) * ST],
                        start=(kb == 0), stop=(kb == nkb - 1))
                if kg == 0:
                    nc.vector.tensor_copy(pa, pt)
                else:
                    nc.vector.tensor_add(pa, pa, pt)
                yield
            pd = self.psX.tile([128, ST], F32, tag="px", name="pd")
            for ph in range(KPG):
                nc.tensor.matmul(pd, lhsT=self.ones_sb,
                                 rhs=pa[:, ph * ST:(ph + 1) * ST],
                                 start=(ph == 0), stop=(ph == KPG - 1))
            rec = self.rcp.tile([128, ST], F32, tag="rec")
            nc.vector.reciprocal_approx_fast(out=rec, in_=pd)
            aot = self.aotp.tile([128, ST], F16, tag="aot")
            nc.vector.tensor_mul(aot, po, rec)
            for ci in range(ST // CSZ):
                r0 = b * S + q0 + ci * CSZ
                nc.sync.dma_start(
                    out=self.a2a_ins[h][r0 // RPC, :,
                                        r0 % RPC:r0 % RPC + CSZ],
                    in_=aot[:, ci * CSZ:(ci + 1) * CSZ])


def _outproj(nc, cfg, wo, a2a_outs, wop, aip, obp, psU, out_rows):
    """out_rows[s', :] = AO^T_full[:, s'].T @ wo  (contraction over heads)."""
    D, HPC, RPC, NHB = cfg.D, cfg.HPC, cfg.RPC, cfg.NHB
    NDC = D // 512

    wo_sb = wop.tile([128, NHB, D], F16)
    nc.sync.dma_start(out=wo_sb,
                      in_=wo[:, :].rearrange("(i p) c -> p i c", p=128))
    aot_sb = []
    for i in range(NHB):
        a = aip.tile([128, RPC], F16, tag=f"ai{i}", name=f"ai_sb{i}")
        nc.sync.dma_start(out=a, in_=a2a_outs[i % HPC][i // HPC, :, :])
        aot_sb.append(a)

    for ss in range(RPC // 128):
        pu = [psU.tile([128, 512], F32, tag="pu", name=f"pu_{dct}")
              for dct in range(NDC)]
        for i in range(NHB):
            for dct in range(NDC):
                nc.tensor.matmul(
                    pu[dct],
                    lhsT=aot_sb[i][:, ss * 128:(ss + 1) * 128],
                    rhs=wo_sb[:, i, dct * 512:(dct + 1) * 512],
                    start=(i == 0), stop=(i == NHB - 1))
        for dct in range(NDC):
            ob = obp.tile([128, 512], F32, tag="ob")
            nc.scalar.copy(out=ob, in_=pu[dct])
            nc.sync.dma_start(
                out=out_rows[ss * 128:(ss + 1) * 128,
                             dct * 512:(dct + 1) * 512],
                in_=ob)


# ---------------------------------------------------------------------------
# Host side
# ---------------------------------------------------------------------------

def _rope_perm(hd):
    return np.concatenate([np.arange(0, hd, 2), np.arange(1, hd, 2)])


def prepare_inputs(cfg: Cfg, x, freq_cis, wq_base, wk_base, wv_base, head_a,
                   head_b, q_a, q_b, k_a, k_b, v_a, v_b, wo):
    """Fold LoRA + softmax scale + RoPE permutation into per-core weights."""
    B, S, D, HD, HPC, NC_ = cfg.B, cfg.S, cfg.D, cfg.HD, cfg.HPC, cfg.NCORES
    HEADS = HPC * NC_
    LORA_SCALE = 2.0
    sm = 1.0 / math.sqrt(HD)

    def fold(w_base, oa, ob):
        w = w_base.astype(np.float64).copy()
        only = LORA_SCALE * (oa.astype(np.float64) @ ob.astype(np.float64))
        hoff = LORA_SCALE * (head_a.astype(np.float64)
                             @ head_b.astype(np.float64))
        w += hoff
        w += np.tile(only, (1, HEADS))
        return w

    wq_eff = fold(wq_base, q_a, q_b) * sm
    wk_eff = fold(wk_base, k_a, k_b)
    wv_eff = fold(wv_base, v_a, v_b)

    perm = _rope_perm(HD)
    for h in range(HEADS):
        cols = h * HD + perm
        wq_eff[:, h * HD:(h + 1) * HD] = wq_eff[:, cols]
        wk_eff[:, h * HD:(h + 1) * HD] = wk_eff[:, cols]
    wq_eff = wq_eff.astype(np.float16)
    wk_eff = wk_eff.astype(np.float16)
    wv_eff = wv_eff.astype(np.float16)

    xT = np.ascontiguousarray(x.reshape(cfg.BS, D).T.astype(np.float16))

    cos = freq_cis[:S, :, 0].T.astype(np.float32)   # [64, S]
    sin = freq_cis[:S, :, 1].T.astype(np.float32)
    cosT = np.ascontiguousarray(np.concatenate([cos, cos], axis=0))
    sinT = np.ascontiguousarray(np.concatenate([sin, sin], axis=0))

    ST = cfg.ST
    ii = np.arange(128)[:, None]
    cc = np.arange(2 * ST - 128)[None, :]
    band = np.where(ii <= cc - (ST - 128), 0.0, NEG).astype(np.float32)

    in_maps = []
    for c in range(NC_):
        sl = slice(c * HPC * HD, (c + 1) * HPC * HD)
        in_maps.append(dict(
            xT=xT,
            wq=np.ascontiguousarray(wq_eff[:, sl]),
            wk=np.ascontiguousarray(wk_eff[:, sl]),
            wv=np.ascontiguousarray(wv_eff[:, sl]),
            wo=np.ascontiguousarray(wo.astype(np.float16)),
            cosT=cosT, sinT=sinT, band=band,
        ))
    return in_maps


_BASS_CACHE = {}


def _get_bass(cfg: Cfg):
    key = (cfg.B, cfg.S, cfg.D, cfg.HPC, cfg.NCORES)
    if key not in _BASS_CACHE:
        _BASS_CACHE[key] = build_bass(cfg)
    return _BASS_CACHE[key]


def kernel(**inputs) -> np.ndarray:
    from concourse.bass_utils import run_bass_kernel_spmd

    x = np.asarray(inputs["x"])
    B, S, D = x.shape
    cfg = Cfg(B=B, S=S, D=D, HEADS=16, NCORES=8)
    in_maps = prepare_inputs(cfg, **{k: np.asarray(v)
                                     for k, v in inputs.items()})
    nc = _get_bass(cfg)
    res = run_bass_kernel_spmd(nc, in_maps, list(range(cfg.NCORES)))
    rows = np.concatenate([res.results[c]["out_rows"]
                           for c in range(cfg.NCORES)], axis=0)
    return rows.reshape(B, S, D).astype(np.float32)
